# revision 27
# baseline (speedup 1.0000x reference)
"""AMS loss kernel for Trainium2, data-parallel over 8 NeuronCores.

Reference computation (per row r of logits [N, C], target t_r):
    num_r   = logits[r, t_r]
    denom_r = exp(num_r) + (sum_j exp(logits[r, j])) * e^M - exp(num_r) * e^M
    L_r     = num_r - log(denom_r + EPS)
    out     = -mean_r(L_r)

Memory-bound problem: the f32 logits stream is the roofline, so the host
casts logits to fp8-e3m4 (1 B/elem, 4 mantissa bits; quantization error on
the final loss is ~1e-6 measured) and the device reads a quarter of the
bytes.  That makes the per-core exp throughput the next wall (ScalarE
activation is 1 elem/cycle/lane = 153.6 G/s vs 20.48 M elem/core), so the
exp+row-sum work is split across three engines:

 - Share A (cols [0, CA)), row-major tiles [128, CA]: ScalarE computes
   exp via activation with fused per-row accumulate (summA[:, j] per tile).
 - Share B (cols [CA, 10000), 49 col-tiles), transposed tiles
   [128 cols, 2048 rows]: DVE computes exp with a Schraudolph bit-trick --
   tensor_scalar(mult, add) producing int16 whose bits are the bf16
   representation of exp(x) -- at the 2x_2P dual-port rate (0.5 cyc/elem).
   The TensorE then row-sums those bf16 tiles with an all-ones stationary
   matmul into PSUM (rows on the free axis, replicated over partitions),
   accumulating all 49 tiles.
 - The PSUM row-sum vector [2048] goes out to a DRAM scratchpad from one
   partition, comes back as a contiguous [16, 128] tile, and a tiny PE
   transpose (identity matmul) turns it into [128, 16]; the [128, 16]
   epilogue then mirrors the reference math exactly.

num_r is gathered on the host (exact f32) and shipped as a [128, 16] input;
the host also sums the 8 partial scalars and scales by -1/N.

Raw Bass (no Tile framework), explicit semaphores per engine.
"""

import sys
import numpy as np

for _p in ("/opt/trn_rl_repo",):
    if _p not in sys.path:
        sys.path.insert(0, _p)

N_TOTAL = 16384
C = 10000
N_CORES = 8
ROWS = N_TOTAL // N_CORES        # 2048 rows per core
P = 128                          # partitions
TILES = ROWS // P                # 16 row-tiles (share A) per core
M = 0.4
EPS = 1e-10

# The B path (DVE+PE) carries a ~10us serial tail (PSUM extraction DRAM
# roundtrip); the A path (ScalarE) has none, so ScalarE gets ~14us more
# streaming work and both paths finish together.
CA = 4112                        # share-A columns (ScalarE)
CB = C - CA                      # 5888 = 46 * 128 (share B, DVE+PE)
NT = CB // P                     # 46 transposed col-tiles
# B chunks: groups of col-tiles processed per DVE instruction; small head
# chunks so the DVE starts as soon as the first quarter-MB lands; small
# tail chunk so the PE finishes right behind the DVE
CHUNK_TILES = [1, 1, 2] + [4] * 10 + [2]
NCH = len(CHUNK_TILES)
NA = 4                           # A tile buffer slots
NB = 3                           # B chunk buffer slots
NY = 2                           # yi16 buffer slots

# Schraudolph constants: int16(x*128/ln2 + (127*128 - 7 + M*128/ln2)) bits
# ~ bf16(exp(x + M))  (the e^M factor of the reference denom is folded in)
S16 = 128.0 / float(np.log(2.0))
C16 = 127.0 * 128.0 - 7.0 + M * 128.0 / float(np.log(2.0))

PROFILE = False                  # set True (e.g. by test.py) to capture NTFF profile
DEBUG = False                    # add intermediate-tensor outputs for debugging
LAST_RESULT = None               # BassKernelResults of the last run (for profiling)

_CACHE = {}


def _build_nc():
    from contextlib import ExitStack

    import concourse.bass as bass
    import concourse.mybir as mybir

    F32 = mybir.dt.float32
    BF16 = mybir.dt.bfloat16
    FP8E3 = mybir.dt.float8e3
    FP8E4 = mybir.dt.float8e4
    I16 = mybir.dt.int16
    U8 = mybir.dt.uint8
    Alu = mybir.AluOpType
    Act = mybir.ActivationFunctionType

    EXP_M = float(np.exp(np.float32(M)))

    CMAX = max(CHUNK_TILES) * ROWS          # 8192
    ch_off = [0] * (NCH + 1)                # cumulative col-tile count
    for g in range(NCH):
        ch_off[g + 1] = ch_off[g] + CHUNK_TILES[g]

    nc = bass.Bass()
    a_pack = nc.declare_dram_parameter("a_pack", [P, TILES * CA], U8, isOutput=False)
    b_pack = nc.declare_dram_parameter("b_pack", [P, NT * ROWS], U8, isOutput=False)
    num_in = nc.declare_dram_parameter("num", [P, TILES], F32, isOutput=False)
    ident_in = nc.declare_dram_parameter("ident", [TILES, TILES], F32, isOutput=False)
    id128_in = nc.declare_dram_parameter("id128", [P, P], F32, isOutput=False)
    out = nc.declare_dram_parameter("out", [P, 1], F32, isOutput=True)
    srow = nc.dram_tensor("srow", [TILES, P], F32, kind="Internal")
    if DEBUG:
        dbg_sa = nc.declare_dram_parameter("dbg_sa", [P, TILES], F32, isOutput=True)
        dbg_st = nc.declare_dram_parameter("dbg_st", [P, TILES], F32, isOutput=True)
        dbg_en = nc.declare_dram_parameter("dbg_en", [P, TILES], F32, isOutput=True)
        dbg_dn = nc.declare_dram_parameter("dbg_dn", [P, TILES], F32, isOutput=True)
        dbg_s16 = nc.declare_dram_parameter("dbg_s16", [TILES, P], F32, isOutput=True)
        dbg_y = nc.declare_dram_parameter("dbg_y", [P, ROWS], mybir.dt.int16, isOutput=True)

    with ExitStack() as ctx:
        en_ctx = ctx.enter_context
        ta = [en_ctx(nc.sbuf_tensor(f"ta{i}", [P, CA], U8)) for i in range(NA)]
        tb = [en_ctx(nc.sbuf_tensor(f"tb{i}", [P, CMAX], U8)) for i in range(NB)]
        yi = [en_ctx(nc.sbuf_tensor(f"yi{i}", [P, CMAX], I16)) for i in range(NY)]
        gact = en_ctx(nc.sbuf_tensor("gact", [P, CA], FP8E4))   # unused act out
        ones_sb = en_ctx(nc.sbuf_tensor("ones", [P, P], BF16))
        ident_sb = en_ctx(nc.sbuf_tensor("ident_sb", [TILES, TILES], F32))
        id128_sb = en_ctx(nc.sbuf_tensor("id128_sb", [P, P], F32))
        bias_m = en_ctx(nc.sbuf_tensor("bias_m", [P, 1], F32))
        num_sb = en_ctx(nc.sbuf_tensor("num_sb", [P, TILES], F32))
        summA = en_ctx(nc.sbuf_tensor("summA", [P, TILES], F32))
        s16 = en_ctx(nc.sbuf_tensor("s16", [TILES, P], F32))
        sumBT = en_ctx(nc.sbuf_tensor("sumBT", [P, TILES], F32))
        en = en_ctx(nc.sbuf_tensor("en", [P, TILES], F32))
        en1 = en_ctx(nc.sbuf_tensor("en1", [P, TILES], F32))
        lnd = en_ctx(nc.sbuf_tensor("lnd", [P, TILES], F32))
        lg = en_ctx(nc.sbuf_tensor("lg", [P, TILES], F32))
        partial = en_ctx(nc.sbuf_tensor("partial", [P, 1], F32))
        bias_eps = en_ctx(nc.sbuf_tensor("bias_eps", [P, 1], F32))
        srow_sb = en_ctx(nc.sbuf_tensor("srow_sb", [1, ROWS], F32))

        psum = en_ctx(nc.psum_tensor("ps", [P, ROWS], F32))
        psum_t = en_ctx(nc.psum_tensor("ps_t", [P, TILES], F32))
        psum_t2 = en_ctx(nc.psum_tensor("ps_t2", [P, TILES], F32))

        n_sem = en_ctx(nc.semaphore("n_sem"))      # num DMA landed
        a_dma = en_ctx(nc.semaphore("a_dma"))      # A tiles landed (16/tile)
        b_dma = en_ctx(nc.semaphore("b_dma"))      # B chunks landed (16/chunk)
        a_cons = en_ctx(nc.semaphore("a_cons"))    # ScalarE consumed A tile
        y_sem = en_ctx(nc.semaphore("y_sem"))      # DVE produced yi16 chunk
        pe_sem = en_ctx(nc.semaphore("pe_sem"))    # PE consumed yi16 chunk
        v_init = en_ctx(nc.semaphore("v_init"))    # ones/bias memsets done
        en_sem = en_ctx(nc.semaphore("en_sem"))    # en = exp(num) done
        ps_sem = en_ctx(nc.semaphore("ps_sem"))    # psum->sbuf copy done
        sr_sem = en_ctx(nc.semaphore("sr_sem"))    # srow -> DRAM done
        s16_sem = en_ctx(nc.semaphore("s16_sem"))  # srow back as [16, 128]
        pt_sem = en_ctx(nc.semaphore("pt_sem"))    # PE denom accumulation done
        tr_sem = en_ctx(nc.semaphore("tr_sem"))    # PE transpose done
        bt_sem = en_ctx(nc.semaphore("bt_sem"))    # sumBT copy done
        d_sem = en_ctx(nc.semaphore("d_sem"))      # DVE lg done
        e1_sem = en_ctx(nc.semaphore("e1_sem"))    # en1 done
        ln_sem = en_ctx(nc.semaphore("ln_sem"))    # Ln done
        out_sem = en_ctx(nc.semaphore("out_sem"))

        block = en_ctx(nc.Block())

        @block.sync
        def _(sync):
            # interleaved A/B stream (A-tiles lead: ScalarE is fed first)
            seq = []
            na_, nb_ = 0, 0
            while na_ < TILES or nb_ < NCH:
                if na_ < TILES:
                    seq.append(("A", na_)); na_ += 1
                if nb_ < NCH:
                    seq.append(("B", nb_)); nb_ += 1
            for kind, idx in seq:
                if kind == "A":
                    j = idx
                    if j >= NA:
                        sync.wait_ge(a_cons, j - NA + 1)
                    sync.dma_start(
                        out=ta[j % NA][:, :], in_=a_pack[:, j * CA : (j + 1) * CA]
                    ).then_inc(a_dma, 16)
                else:
                    g = idx
                    w = CHUNK_TILES[g] * ROWS
                    lo = ch_off[g] * ROWS
                    if g >= NB:
                        sync.wait_ge(y_sem, g - NB + 1)
                    sync.dma_start(
                        out=tb[g % NB][:, :w], in_=b_pack[:, lo : lo + w]
                    ).then_inc(b_dma, 16)
            sync.wait_ge(d_sem, 1)
            sync.dma_start(out=out[:], in_=partial[:]).then_inc(out_sem, 16)
            if DEBUG:
                sync.dma_start(out=dbg_sa[:], in_=summA[:, :]).then_inc(out_sem, 16)
                sync.dma_start(out=dbg_st[:], in_=lg[:, :]).then_inc(out_sem, 16)
                sync.dma_start(out=dbg_en[:], in_=en[:, :]).then_inc(out_sem, 16)
                sync.dma_start(out=dbg_dn[:], in_=lnd[:, :]).then_inc(out_sem, 16)
                sync.dma_start(out=dbg_s16[:], in_=s16[:, :]).then_inc(out_sem, 16)
                sync.dma_start(out=dbg_y[:], in_=yi[0][:, :ROWS]).then_inc(out_sem, 16)

        @block.gpsimd
        def _(gpsimd):
            gpsimd.dma_start(out=num_sb[:, :], in_=num_in[:, :]).then_inc(n_sem, 16)
            gpsimd.dma_start(out=ident_sb[:, :], in_=ident_in[:, :]).then_inc(n_sem, 16)
            gpsimd.dma_start(out=id128_sb[:, :], in_=id128_in[:, :]).then_inc(n_sem, 16)
            # srow roundtrip: PSUM row-sums -> DRAM -> [16, 128] (contiguous);
            # on the SWDGE queue so it does not sit behind the SP stream DMAs
            gpsimd.wait_ge(ps_sem, 1)
            gpsimd.dma_start(
                out=srow.rearrange("j p -> () (j p)"), in_=srow_sb[:, :]
            ).then_inc(sr_sem, 16)
            gpsimd.wait_ge(sr_sem, 16)
            gpsimd.dma_start(out=s16[:, :], in_=srow[:, :]).then_inc(s16_sem, 16)

        @block.vector
        def _(vector):
            vector.memset(ones_sb[:, :], 1.0).then_inc(v_init, 1)
            vector.memset(bias_eps[:], EPS).then_inc(v_init, 1)
            vector.memset(bias_m[:], M).then_inc(v_init, 1)
            for g in range(NCH):
                w = CHUNK_TILES[g] * ROWS
                vector.wait_ge(b_dma, 16 * (g + 1))
                if g >= NY:
                    vector.wait_ge(pe_sem, g - NY + 1)
                vector.tensor_scalar(
                    out=yi[g % NY][:, :w],
                    in0=tb[g % NB][:, :w].bitcast(FP8E3),
                    scalar1=S16,
                    scalar2=C16,
                    op0=Alu.mult,
                    op1=Alu.add,
                ).then_inc(y_sem, 1)
                if g == 7:
                    # en1 = exp(num) * (1 - e^M), computed mid-stream
                    vector.wait_ge(en_sem, 1)
                    vector.tensor_scalar(
                        out=en1[:, :], in0=en[:, :], scalar1=1.0 - EXP_M,
                        scalar2=None, op0=Alu.mult,
                    ).then_inc(e1_sem, 1)
            # PSUM row-sums (replicated over partitions): partition 0 -> SBUF
            vector.wait_ge(pe_sem, NCH)
            vector.tensor_copy(srow_sb[:, :], psum[0:1, :]).then_inc(ps_sem, 1)
            vector.wait_ge(tr_sem, 1)
            vector.tensor_copy(sumBT[:, :], psum_t2[:, :]).then_inc(bt_sem, 1)
            # epilogue
            vector.wait_ge(ln_sem, 1)
            vector.scalar_tensor_tensor(
                out=lg[:, :],
                in0=num_sb[:, :],
                scalar=1.0,
                in1=lnd[:, :],
                op0=Alu.mult,
                op1=Alu.subtract,
                accum_out=partial[:],
            ).then_inc(d_sem, 1)

        @block.scalar
        def _(scalar):
            scalar.wait_ge(v_init, 3)
            for j in range(TILES):
                scalar.wait_ge(a_dma, 16 * (j + 1))
                scalar.activation(
                    out=gact[:, :],
                    in_=ta[j % NA][:, :].bitcast(FP8E3),
                    func=Act.Exp,
                    bias=bias_m[:],
                    accum_out=summA[:, j : j + 1],
                ).then_inc(a_cons, 1)
                if j == 8:
                    scalar.wait_ge(n_sem, 16)
                    scalar.activation(
                        out=en[:, :], in_=num_sb[:, :], func=Act.Exp
                    ).then_inc(en_sem, 1)
            scalar.wait_ge(pt_sem, 1)
            scalar.activation(
                out=lnd[:, :], in_=psum_t[:, :], func=Act.Ln, bias=bias_eps[:]
            ).then_inc(ln_sem, 1)

        @block.tensor
        def _(tensor):
            first_q = {}
            last_q = {}
            for g in range(NCH):
                for s in range(CHUNK_TILES[g] * ROWS // 512):
                    first_q.setdefault(s % 4, (g, s))
                    last_q[s % 4] = (g, s)
            tensor.wait_ge(v_init, 1)
            for g in range(NCH):
                w = CHUNK_TILES[g] * ROWS
                nsub = w // 512
                tensor.wait_ge(y_sem, g + 1)
                for s in range(nsub):
                    q = s % 4
                    mm = tensor.matmul(
                        out=psum[:, q * 512 : (q + 1) * 512],
                        lhsT=ones_sb[:, :],
                        rhs=yi[g % NY][:, s * 512 : (s + 1) * 512].bitcast(BF16),
                        start=(first_q[q] == (g, s)),
                        stop=(last_q[q] == (g, s)),
                    )
                    if s == nsub - 1:
                        mm.then_inc(pe_sem, 1)
            # denom accumulates in psum_t: s16.T (share-B row-sums, e^M
            # folded) + summA (share-A, e^M folded) + en1 = exp(num)(1-e^M)
            tensor.wait_ge(n_sem, 48)
            tensor.wait_ge(s16_sem, 16)
            tensor.transpose(
                out=psum_t2[:, :], in_=s16[:, :], identity=ident_sb[:, :]
            ).then_inc(tr_sem, 1)
            tensor.wait_ge(e1_sem, 1)
            tensor.wait_ge(bt_sem, 1)
            tensor.matmul(
                out=psum_t[:, :], lhsT=id128_sb[:, :], rhs=en1[:, :],
                start=True, stop=False,
            )
            tensor.matmul(
                out=psum_t[:, :], lhsT=id128_sb[:, :], rhs=sumBT[:, :],
                start=False, stop=False,
            )
            tensor.wait_ge(a_cons, TILES)
            tensor.matmul(
                out=psum_t[:, :], lhsT=id128_sb[:, :], rhs=summA[:, :],
                start=False, stop=True,
            ).then_inc(pt_sem, 1)

    return nc


def _get_nc():
    if "nc" not in _CACHE:
        _CACHE["nc"] = _build_nc()
    return _CACHE["nc"]


def kernel(logits, targets):
    global LAST_RESULT
    import ml_dtypes
    from concourse.bass_utils import run_bass_kernel_spmd

    logits = np.ascontiguousarray(np.asarray(logits), dtype=np.float32)
    targets = np.asarray(targets).astype(np.int64)
    assert logits.shape == (N_TOTAL, C), logits.shape
    assert targets.shape == (N_TOTAL,), targets.shape

    # exact f32 target logits, laid out [128, 16]: (p, j) <-> row 128j + p
    num_full = logits[np.arange(N_TOTAL), targets].astype(np.float32)
    # fp8 e3m4 cast of the full logits (bytes shipped to the device)
    l8 = logits.astype(ml_dtypes.float8_e3m4).view(np.uint8)

    in_maps = []
    for k in range(N_CORES):
        lo, hi = k * ROWS, (k + 1) * ROWS
        shard = l8[lo:hi]
        a = np.ascontiguousarray(
            shard[:, :CA].reshape(TILES, P, CA).transpose(1, 0, 2).reshape(P, -1)
        )
        b = np.ascontiguousarray(
            shard[:, CA:].T.reshape(NT, P, ROWS).transpose(1, 0, 2).reshape(P, -1)
        )
        nm = np.ascontiguousarray(num_full[lo:hi].reshape(TILES, P).T)
        in_maps.append(
            {"a_pack": a, "b_pack": b, "num": nm,
             "ident": np.eye(TILES, dtype=np.float32),
             "id128": np.eye(P, dtype=np.float32)}
        )

    nc = _get_nc()
    result = run_bass_kernel_spmd(
        nc, in_maps, core_ids=list(range(N_CORES)), trace=PROFILE
    )
    LAST_RESULT = result
    total = np.float64(0.0)
    for r in result.results:
        total += np.float64(r["out"].sum())
    return np.float32(-total / N_TOTAL)


# revision 28
# speedup vs baseline: 1.2076x; 1.2076x over previous
"""AMS loss kernel for Trainium2, data-parallel over 8 NeuronCores.

Reference computation (per row r of logits [N, C], target t_r):
    num_r   = logits[r, t_r]
    denom_r = exp(num_r) + (sum_j exp(logits[r, j])) * e^M - exp(num_r) * e^M
    L_r     = num_r - log(denom_r + EPS)
    out     = -mean_r(L_r)

Memory-bound problem: the f32 logits stream is the roofline, so the host
casts logits to fp8-e3m4 (1 B/elem, 4 mantissa bits; quantization error on
the final loss is ~1e-6 measured) and the device reads a quarter of the
bytes.  That makes the per-core exp throughput the next wall (ScalarE
activation is 1 elem/cycle/lane = 153.6 G/s vs 20.48 M elem/core), so the
exp+row-sum work is split across three engines:

 - Share A (cols [0, CA)), row-major tiles [128, CA]: ScalarE computes
   exp via activation with fused per-row accumulate (summA[:, j] per tile).
 - Share B (cols [CA, 10000), 49 col-tiles), transposed tiles
   [128 cols, 2048 rows]: DVE computes exp with a Schraudolph bit-trick --
   tensor_scalar(mult, add) producing int16 whose bits are the bf16
   representation of exp(x) -- at the 2x_2P dual-port rate (0.5 cyc/elem).
   The TensorE then row-sums those bf16 tiles with an all-ones stationary
   matmul into PSUM (rows on the free axis, replicated over partitions),
   accumulating all 49 tiles.
 - The PSUM row-sum vector [2048] goes out to a DRAM scratchpad from one
   partition, comes back as a contiguous [16, 128] tile, and a tiny PE
   transpose (identity matmul) turns it into [128, 16]; the [128, 16]
   epilogue then mirrors the reference math exactly.

num_r is gathered on the host (exact f32) and shipped as a [128, 16] input;
the host also sums the 8 partial scalars and scales by -1/N.

Raw Bass (no Tile framework), explicit semaphores per engine.
"""

import sys
import numpy as np

for _p in ("/opt/trn_rl_repo",):
    if _p not in sys.path:
        sys.path.insert(0, _p)

N_TOTAL = 16384
C = 10000
N_CORES = 8
ROWS = N_TOTAL // N_CORES        # 2048 rows per core
P = 128                          # partitions
TILES = ROWS // P                # 16 row-tiles (share A) per core
M = 0.4
EPS = 1e-10

# The B path (DVE+PE) carries a ~10us serial tail (PSUM extraction DRAM
# roundtrip); the A path (ScalarE) has none, so ScalarE gets ~14us more
# streaming work and both paths finish together.
CA = 4240                        # share-A columns (ScalarE)
CB = C - CA                      # 5760 = 45 * 128 (share B, DVE+PE)
NT = CB // P                     # 45 transposed col-tiles
# B chunks: groups of col-tiles processed per DVE instruction; small head
# chunks so the DVE starts as soon as the first quarter-MB lands; small
# tail chunk so the PE finishes right behind the DVE
CHUNK_TILES = [1, 1, 2] + [4] * 10 + [1]
NCH = len(CHUNK_TILES)
NA = 4                           # A tile buffer slots
NB = 3                           # B chunk buffer slots
NY = 2                           # yi16 buffer slots

# Schraudolph constants: int16(x*128/ln2 + (127*128 - 7 + M*128/ln2)) bits
# ~ bf16(exp(x + M))  (the e^M factor of the reference denom is folded in)
S16 = 128.0 / float(np.log(2.0))
C16 = 127.0 * 128.0 - 7.0 + M * 128.0 / float(np.log(2.0))

PROFILE = False                  # set True (e.g. by test.py) to capture NTFF profile
DEBUG = False                    # add intermediate-tensor outputs for debugging
LAST_RESULT = None               # BassKernelResults of the last run (for profiling)

_CACHE = {}


def _build_nc():
    from contextlib import ExitStack

    import concourse.bass as bass
    import concourse.mybir as mybir

    F32 = mybir.dt.float32
    BF16 = mybir.dt.bfloat16
    FP8E3 = mybir.dt.float8e3
    FP8E4 = mybir.dt.float8e4
    I16 = mybir.dt.int16
    U8 = mybir.dt.uint8
    Alu = mybir.AluOpType
    Act = mybir.ActivationFunctionType

    EXP_M = float(np.exp(np.float32(M)))

    CMAX = max(CHUNK_TILES) * ROWS          # 8192
    ch_off = [0] * (NCH + 1)                # cumulative col-tile count
    for g in range(NCH):
        ch_off[g + 1] = ch_off[g] + CHUNK_TILES[g]

    nc = bass.Bass()
    a_pack = nc.declare_dram_parameter("a_pack", [P, TILES * CA], U8, isOutput=False)
    b_pack = nc.declare_dram_parameter("b_pack", [P, NT * ROWS], U8, isOutput=False)
    num_in = nc.declare_dram_parameter("num", [P, TILES], F32, isOutput=False)
    ident_in = nc.declare_dram_parameter("ident", [TILES, TILES], F32, isOutput=False)
    id128_in = nc.declare_dram_parameter("id128", [P, P], F32, isOutput=False)
    out = nc.declare_dram_parameter("out", [P, 1], F32, isOutput=True)
    srow = nc.dram_tensor("srow", [TILES, P], F32, kind="Internal")
    if DEBUG:
        dbg_sa = nc.declare_dram_parameter("dbg_sa", [P, TILES], F32, isOutput=True)
        dbg_st = nc.declare_dram_parameter("dbg_st", [P, TILES], F32, isOutput=True)
        dbg_en = nc.declare_dram_parameter("dbg_en", [P, TILES], F32, isOutput=True)
        dbg_dn = nc.declare_dram_parameter("dbg_dn", [P, TILES], F32, isOutput=True)
        dbg_s16 = nc.declare_dram_parameter("dbg_s16", [TILES, P], F32, isOutput=True)
        dbg_y = nc.declare_dram_parameter("dbg_y", [P, ROWS], mybir.dt.int16, isOutput=True)

    with ExitStack() as ctx:
        en_ctx = ctx.enter_context
        ta = [en_ctx(nc.sbuf_tensor(f"ta{i}", [P, CA], U8)) for i in range(NA)]
        tb = [en_ctx(nc.sbuf_tensor(f"tb{i}", [P, CMAX], U8)) for i in range(NB)]
        yi = [en_ctx(nc.sbuf_tensor(f"yi{i}", [P, CMAX], I16)) for i in range(NY)]
        gact = en_ctx(nc.sbuf_tensor("gact", [P, CA], FP8E4))   # unused act out
        ones_sb = en_ctx(nc.sbuf_tensor("ones", [P, P], BF16))
        ident_sb = en_ctx(nc.sbuf_tensor("ident_sb", [TILES, TILES], F32))
        id128_sb = en_ctx(nc.sbuf_tensor("id128_sb", [P, P], F32))
        bias_m = en_ctx(nc.sbuf_tensor("bias_m", [P, 1], F32))
        num_sb = en_ctx(nc.sbuf_tensor("num_sb", [P, TILES], F32))
        summA = en_ctx(nc.sbuf_tensor("summA", [P, TILES], F32))
        s16 = en_ctx(nc.sbuf_tensor("s16", [TILES, P], F32))
        sumBT = en_ctx(nc.sbuf_tensor("sumBT", [P, TILES], F32))
        en = en_ctx(nc.sbuf_tensor("en", [P, TILES], F32))
        en1 = en_ctx(nc.sbuf_tensor("en1", [P, TILES], F32))
        lnd = en_ctx(nc.sbuf_tensor("lnd", [P, TILES], F32))
        lg = en_ctx(nc.sbuf_tensor("lg", [P, TILES], F32))
        partial = en_ctx(nc.sbuf_tensor("partial", [P, 1], F32))
        bias_eps = en_ctx(nc.sbuf_tensor("bias_eps", [P, 1], F32))
        srow_sb = en_ctx(nc.sbuf_tensor("srow_sb", [1, ROWS], F32))

        psum = en_ctx(nc.psum_tensor("ps", [P, ROWS], F32))
        psum_t = en_ctx(nc.psum_tensor("ps_t", [P, TILES], F32))
        psum_t2 = en_ctx(nc.psum_tensor("ps_t2", [P, TILES], F32))

        n_sem = en_ctx(nc.semaphore("n_sem"))      # num DMA landed
        a_dma = en_ctx(nc.semaphore("a_dma"))      # A tiles landed (16/tile)
        b_dma = en_ctx(nc.semaphore("b_dma"))      # B chunks landed (16/chunk)
        a_cons = en_ctx(nc.semaphore("a_cons"))    # ScalarE consumed A tile
        y_sem = en_ctx(nc.semaphore("y_sem"))      # DVE produced yi16 chunk
        pe_sem = en_ctx(nc.semaphore("pe_sem"))    # PE consumed yi16 chunk
        v_init = en_ctx(nc.semaphore("v_init"))    # ones/bias memsets done
        en_sem = en_ctx(nc.semaphore("en_sem"))    # en = exp(num) done
        ps_sem = en_ctx(nc.semaphore("ps_sem"))    # psum->sbuf copy done
        sr_sem = en_ctx(nc.semaphore("sr_sem"))    # srow -> DRAM done
        s16_sem = en_ctx(nc.semaphore("s16_sem"))  # srow back as [16, 128]
        pt_sem = en_ctx(nc.semaphore("pt_sem"))    # PE denom accumulation done
        tr_sem = en_ctx(nc.semaphore("tr_sem"))    # PE transpose done
        bt_sem = en_ctx(nc.semaphore("bt_sem"))    # sumBT copy done
        d_sem = en_ctx(nc.semaphore("d_sem"))      # DVE lg done
        e1_sem = en_ctx(nc.semaphore("e1_sem"))    # en1 done
        ln_sem = en_ctx(nc.semaphore("ln_sem"))    # Ln done
        out_sem = en_ctx(nc.semaphore("out_sem"))

        block = en_ctx(nc.Block())

        @block.sync
        def _(sync):
            # interleaved A/B stream (A-tiles lead: ScalarE is fed first)
            seq = []
            na_, nb_ = 0, 0
            while na_ < TILES or nb_ < NCH:
                if na_ < TILES:
                    seq.append(("A", na_)); na_ += 1
                if nb_ < NCH:
                    seq.append(("B", nb_)); nb_ += 1
            for kind, idx in seq:
                if kind == "A":
                    j = idx
                    if j >= NA:
                        sync.wait_ge(a_cons, j - NA + 1)
                    sync.dma_start(
                        out=ta[j % NA][:, :], in_=a_pack[:, j * CA : (j + 1) * CA]
                    ).then_inc(a_dma, 16)
                else:
                    g = idx
                    w = CHUNK_TILES[g] * ROWS
                    lo = ch_off[g] * ROWS
                    if g >= NB:
                        sync.wait_ge(y_sem, g - NB + 1)
                    sync.dma_start(
                        out=tb[g % NB][:, :w], in_=b_pack[:, lo : lo + w]
                    ).then_inc(b_dma, 16)
            # srow roundtrip: PSUM row-sums -> DRAM -> [16, 128] (contiguous)
            sync.wait_ge(ps_sem, 1)
            sync.dma_start(
                out=srow.rearrange("j p -> () (j p)"), in_=srow_sb[:, :]
            ).then_inc(sr_sem, 16)
            sync.wait_ge(sr_sem, 16)
            sync.dma_start(out=s16[:, :], in_=srow[:, :]).then_inc(s16_sem, 16)
            sync.wait_ge(d_sem, 1)
            sync.dma_start(out=out[:], in_=partial[:]).then_inc(out_sem, 16)
            if DEBUG:
                sync.dma_start(out=dbg_sa[:], in_=summA[:, :]).then_inc(out_sem, 16)
                sync.dma_start(out=dbg_st[:], in_=lg[:, :]).then_inc(out_sem, 16)
                sync.dma_start(out=dbg_en[:], in_=en[:, :]).then_inc(out_sem, 16)
                sync.dma_start(out=dbg_dn[:], in_=lnd[:, :]).then_inc(out_sem, 16)
                sync.dma_start(out=dbg_s16[:], in_=s16[:, :]).then_inc(out_sem, 16)
                sync.dma_start(out=dbg_y[:], in_=yi[0][:, :ROWS]).then_inc(out_sem, 16)

        @block.gpsimd
        def _(gpsimd):
            gpsimd.dma_start(out=num_sb[:, :], in_=num_in[:, :]).then_inc(n_sem, 16)
            gpsimd.dma_start(out=ident_sb[:, :], in_=ident_in[:, :]).then_inc(n_sem, 16)
            gpsimd.dma_start(out=id128_sb[:, :], in_=id128_in[:, :]).then_inc(n_sem, 16)

        @block.vector
        def _(vector):
            vector.memset(ones_sb[:, :], 1.0).then_inc(v_init, 1)
            vector.memset(bias_eps[:], EPS).then_inc(v_init, 1)
            vector.memset(bias_m[:], M).then_inc(v_init, 1)
            for g in range(NCH):
                w = CHUNK_TILES[g] * ROWS
                vector.wait_ge(b_dma, 16 * (g + 1))
                if g >= NY:
                    vector.wait_ge(pe_sem, g - NY + 1)
                vector.tensor_scalar(
                    out=yi[g % NY][:, :w],
                    in0=tb[g % NB][:, :w].bitcast(FP8E3),
                    scalar1=S16,
                    scalar2=C16,
                    op0=Alu.mult,
                    op1=Alu.add,
                ).then_inc(y_sem, 1)
                if g == 7:
                    # en1 = exp(num) * (1 - e^M), computed mid-stream
                    vector.wait_ge(en_sem, 1)
                    vector.tensor_scalar(
                        out=en1[:, :], in0=en[:, :], scalar1=1.0 - EXP_M,
                        scalar2=None, op0=Alu.mult,
                    ).then_inc(e1_sem, 1)
            # PSUM row-sums (replicated over partitions): partition 0 -> SBUF
            vector.wait_ge(pe_sem, NCH)
            vector.tensor_copy(srow_sb[:, :], psum[0:1, :]).then_inc(ps_sem, 1)
            vector.wait_ge(tr_sem, 1)
            vector.tensor_copy(sumBT[:, :], psum_t2[:, :]).then_inc(bt_sem, 1)
            # epilogue
            vector.wait_ge(ln_sem, 1)
            vector.scalar_tensor_tensor(
                out=lg[:, :],
                in0=num_sb[:, :],
                scalar=1.0,
                in1=lnd[:, :],
                op0=Alu.mult,
                op1=Alu.subtract,
                accum_out=partial[:],
            ).then_inc(d_sem, 1)

        @block.scalar
        def _(scalar):
            scalar.wait_ge(v_init, 3)
            for j in range(TILES):
                scalar.wait_ge(a_dma, 16 * (j + 1))
                scalar.activation(
                    out=gact[:, :],
                    in_=ta[j % NA][:, :].bitcast(FP8E3),
                    func=Act.Exp,
                    bias=bias_m[:],
                    accum_out=summA[:, j : j + 1],
                ).then_inc(a_cons, 1)
                if j == 8:
                    scalar.wait_ge(n_sem, 16)
                    scalar.activation(
                        out=en[:, :], in_=num_sb[:, :], func=Act.Exp
                    ).then_inc(en_sem, 1)
            scalar.wait_ge(pt_sem, 1)
            scalar.activation(
                out=lnd[:, :], in_=psum_t[:, :], func=Act.Ln, bias=bias_eps[:]
            ).then_inc(ln_sem, 1)

        @block.tensor
        def _(tensor):
            first_q = {}
            last_q = {}
            for g in range(NCH):
                for s in range(CHUNK_TILES[g] * ROWS // 512):
                    first_q.setdefault(s % 4, (g, s))
                    last_q[s % 4] = (g, s)
            tensor.wait_ge(v_init, 1)
            for g in range(NCH):
                w = CHUNK_TILES[g] * ROWS
                nsub = w // 512
                tensor.wait_ge(y_sem, g + 1)
                for s in range(nsub):
                    q = s % 4
                    mm = tensor.matmul(
                        out=psum[:, q * 512 : (q + 1) * 512],
                        lhsT=ones_sb[:, :],
                        rhs=yi[g % NY][:, s * 512 : (s + 1) * 512].bitcast(BF16),
                        start=(first_q[q] == (g, s)),
                        stop=(last_q[q] == (g, s)),
                    )
                    if s == nsub - 1:
                        mm.then_inc(pe_sem, 1)
            # denom accumulates in psum_t: s16.T (share-B row-sums, e^M
            # folded) + summA (share-A, e^M folded) + en1 = exp(num)(1-e^M)
            tensor.wait_ge(n_sem, 48)
            tensor.wait_ge(s16_sem, 16)
            tensor.transpose(
                out=psum_t2[:, :], in_=s16[:, :], identity=ident_sb[:, :]
            ).then_inc(tr_sem, 1)
            tensor.wait_ge(e1_sem, 1)
            tensor.wait_ge(bt_sem, 1)
            tensor.matmul(
                out=psum_t[:, :], lhsT=id128_sb[:, :], rhs=en1[:, :],
                start=True, stop=False,
            )
            tensor.matmul(
                out=psum_t[:, :], lhsT=id128_sb[:, :], rhs=sumBT[:, :],
                start=False, stop=False,
            )
            tensor.wait_ge(a_cons, TILES)
            tensor.matmul(
                out=psum_t[:, :], lhsT=id128_sb[:, :], rhs=summA[:, :],
                start=False, stop=True,
            ).then_inc(pt_sem, 1)

    return nc


def _get_nc():
    if "nc" not in _CACHE:
        _CACHE["nc"] = _build_nc()
    return _CACHE["nc"]


def kernel(logits, targets):
    global LAST_RESULT
    import ml_dtypes
    from concourse.bass_utils import run_bass_kernel_spmd

    logits = np.ascontiguousarray(np.asarray(logits), dtype=np.float32)
    targets = np.asarray(targets).astype(np.int64)
    assert logits.shape == (N_TOTAL, C), logits.shape
    assert targets.shape == (N_TOTAL,), targets.shape

    # exact f32 target logits, laid out [128, 16]: (p, j) <-> row 128j + p
    num_full = logits[np.arange(N_TOTAL), targets].astype(np.float32)
    # fp8 e3m4 cast of the full logits (bytes shipped to the device)
    l8 = logits.astype(ml_dtypes.float8_e3m4).view(np.uint8)

    in_maps = []
    for k in range(N_CORES):
        lo, hi = k * ROWS, (k + 1) * ROWS
        shard = l8[lo:hi]
        a = np.ascontiguousarray(
            shard[:, :CA].reshape(TILES, P, CA).transpose(1, 0, 2).reshape(P, -1)
        )
        b = np.ascontiguousarray(
            shard[:, CA:].T.reshape(NT, P, ROWS).transpose(1, 0, 2).reshape(P, -1)
        )
        nm = np.ascontiguousarray(num_full[lo:hi].reshape(TILES, P).T)
        in_maps.append(
            {"a_pack": a, "b_pack": b, "num": nm,
             "ident": np.eye(TILES, dtype=np.float32),
             "id128": np.eye(P, dtype=np.float32)}
        )

    nc = _get_nc()
    result = run_bass_kernel_spmd(
        nc, in_maps, core_ids=list(range(N_CORES)), trace=PROFILE
    )
    LAST_RESULT = result
    total = np.float64(0.0)
    for r in result.results:
        total += np.float64(r["out"].sum())
    return np.float32(-total / N_TOTAL)
